# revision 25
# baseline (speedup 1.0000x reference)
"""Trainium2 Bass kernel for DisentangledSelfAttention (8-core data parallel).

Math (from the reference):
  Q = query @ Wq + bq ; K = key @ Wk + bk ; V = value @ Wv + bv   (per-head split)
  Qc = Q - mean_fields(Q) ; Kc = K - mean_fields(K)               (bq/bk cancel)
  pairwise = softmax(Qc Kc^T)  per (batch, head)
  unary    = softmax over a size-1 axis == 1 everywhere, so
  out = relu((pairwise + 1) @ V + query)
      = relu(pairwise @ V + colsum(V) + query)

Sharding: batch (2048) split across 8 cores, 256 batches/core. Weights are
replicated. Each core streams its 16384x512 row-block in 32 blocks of 512
rows (8 batches).

Layouts per core: query/key/value are fed pre-transposed ([512, 16384],
contraction dim on partitions, fp16) so the three projections run with the
weights stationary; Q/K come out transposed ([A, m] — what the per-head
QK^T matmuls want), V natural ([m, A]) for PV. All matmuls are fp16 with
fp32 PSUM accumulation; every stationary operand spans the full 128
partition rows (zero-padded block-diagonal layouts for the per-head
slices — sub-row stationaries fault on this toolchain).
"""

import sys
from contextlib import ExitStack

sys.path.insert(0, "/opt/trn_rl_repo")

import numpy as np

import concourse.bacc as bacc
import concourse.tile as tile
from concourse import mybir

B, F, D = 2048, 64, 512
A, H, HD = 512, 8, 64
NCORES = 8
BL = B // NCORES          # batches per core
M = BL * F                # rows per core
MB = 512                  # rows per block (8 batches)
NB_FULL = M // MB         # 32 blocks

F32 = mybir.dt.float32
F16 = mybir.dt.float16
AF = mybir.ActivationFunctionType


def bcast_inner(ap2d, inner):
    """[P, n] -> [P, n, inner] with stride-0 inner axis."""
    return ap2d.rearrange("p (b x) -> p b x", x=1).broadcast_to(
        [ap2d.shape[0], ap2d.shape[1], inner]
    )


def build_program(nblocks=NB_FULL, stage=6):
    nc = bacc.Bacc("TRN2", target_bir_lowering=False, debug=False,
                   num_devices=NCORES)
    m_tot = nblocks * MB

    qT = nc.dram_tensor("qT", [D, m_tot], F16, kind="ExternalInput").ap()
    kT = nc.dram_tensor("kT", [D, m_tot], F16, kind="ExternalInput").ap()
    vT = nc.dram_tensor("vT", [D, m_tot], F16, kind="ExternalInput").ap()
    qn = nc.dram_tensor("qn", [m_tot, D], F16, kind="ExternalInput").ap()
    wq = nc.dram_tensor("wq", [D, A], F16, kind="ExternalInput").ap()
    wk = nc.dram_tensor("wk", [D, A], F16, kind="ExternalInput").ap()
    wv = nc.dram_tensor("wv", [D, A], F16, kind="ExternalInput").ap()
    bcast2 = nc.dram_tensor("bcast2", [128, 128], F16,
                            kind="ExternalInput").ap()
    out = nc.dram_tensor("out", [m_tot, A], F16, kind="ExternalOutput").ap()

    with tile.TileContext(nc) as tc, ExitStack() as ctx:
        const = ctx.enter_context(tc.tile_pool(name="const", bufs=1))
        p_in = ctx.enter_context(tc.tile_pool(name="p_in", bufs=3))
        p_stat = ctx.enter_context(tc.tile_pool(name="p_stat", bufs=2))
        p_act = ctx.enter_context(tc.tile_pool(name="p_act", bufs=2))
        p_pt = ctx.enter_context(tc.tile_pool(name="p_pt", bufs=3))
        p_fin = ctx.enter_context(tc.tile_pool(name="p_fin", bufs=2))
        ps_a = ctx.enter_context(tc.tile_pool(name="ps_a", bufs=4, space="PSUM"))
        ps_o = ctx.enter_context(tc.tile_pool(name="ps_o", bufs=2, space="PSUM"))
        

        # --- constants ---
        w_sb = {}
        for name, ap in (("q", wq), ("k", wk), ("v", wv)):
            t = const.tile([128, 4 * A], F16, tag=f"w{name}")
            for kc in range(4):
                nc.sync.dma_start(t[:, kc * A:(kc + 1) * A],
                                  ap[kc * 128:(kc + 1) * 128, :])
            w_sb[name] = t
        bcast2_sb = const.tile([128, 128], F16, tag="bcast2")
        nc.sync.dma_start(bcast2_sb[:], bcast2[:])
        neg8_sb = const.tile([128, 1], F32, tag="neg8")
        nc.vector.memset(neg8_sb[:], -8.0)
        zero_sb = const.tile([128, A], F16, tag="zeroA")
        nc.vector.memset(zero_sb[:], 0.0)

        kc_ring = []
        for r in range(2):
            row = []
            for fc in range(4):
                t = const.tile([128, 2 * MB], F16, tag=f"kc{r}{fc}")
                nc.gpsimd.memset(
                    t[0:64, :].rearrange("p (b c) -> p b c", c=128)[:, :, 64:128],
                    0.0)
                nc.gpsimd.memset(
                    t[64:128, :].rearrange("p (b c) -> p b c", c=128)[:, :, 0:64],
                    0.0)
                row.append(t)
            kc_ring.append(row)
        pt_ring = []
        for r in range(3):
            t = const.tile([128, 8 * 128], F16, tag=f"ptr{r}")
            nc.gpsimd.memset(
                t[0:64, :].rearrange("p (h c) -> p h c", c=128)[:, :, 64:128],
                0.0)
            nc.gpsimd.memset(
                t[64:128, :].rearrange("p (h c) -> p h c", c=128)[:, :, 0:64],
                0.0)
            pt_ring.append(t)
        v16_ring = []
        for r in range(2):
            row = []
            for mt in range(4):
                t = const.tile([128, H * 65], F16, tag=f"v16r{r}{mt}")
                nc.gpsimd.memset(
                    t[:].rearrange("p (h c) -> p h c", c=65)[:, :, 64:65], 1.0)
                row.append(t)
            v16_ring.append(row)

        def emit_dmas(bi):
            m0 = bi * MB
            xc = {}
            big = {}
            for name, src in (("q", qT), ("k", kT), ("v", vT)):
                t = p_in.tile([128, 4 * MB], F16, tag=f"{name}T")
                nc.sync.dma_start(
                    t[:].rearrange("p (x m) -> p x m", m=MB),
                    src.rearrange("(x p) m -> p x m", p=128)[:, :, m0:m0 + MB])
                big[name] = t
            xc["q"] = [big["q"][:, pt * MB:(pt + 1) * MB] for pt in range(4)]
            xc["k"] = [big["k"][:, pt * MB:(pt + 1) * MB] for pt in range(4)]
            vT_t = [big["v"][:, pt * MB:(pt + 1) * MB] for pt in range(4)]
            qn_big = p_in.tile([128, 4 * D], F16, tag="qn")
            nc.sync.dma_start(
                qn_big[:].rearrange("p (x d) -> p x d", d=D),
                qn.rearrange("(x p) d -> p x d", p=128)[:, m0 // 128:
                                                        m0 // 128 + 4, :])
            qn_t = [qn_big[:, mt * D:(mt + 1) * D] for mt in range(4)]
            return dict(bi=bi, m0=m0, xc=xc, vT_t=vT_t, qn_t=qn_t,
                        proj16={"q": [], "k": []}, v16_t=[])

        def proj_units(st):
            """12 closures: Q/K projection f-tiles and V m-tiles. Q -> dense
            fp16 [A-tile, MB]; K -> fp16 block-diagonal per (batch,
            head-parity) so attention stationaries span all 128 rows.
            Field-mean centering is applied post-projection (linear)."""
            bi, xc = st["bi"], st["xc"]

            def qk_unit(name, fc):
                # Q is pre-centered host-side (qT holds query - field-mean).
                # K needs no centering at all: Qc.(K-muK) differs from Qc.K
                # by a per-q constant, cancelled by the softmax over k.
                def emit():
                    ps = ps_a.tile([128, MB], F32, tag="psA")
                    for kc in range(4):
                        nc.tensor.matmul(
                            ps[:],
                            w_sb[name][:, kc * A + fc * 128:
                                       kc * A + fc * 128 + 128],
                            xc[name][kc][:],
                            start=(kc == 0), stop=(kc == 3))
                    if name == "q":
                        t16 = p_act.tile([128, MB], F16, tag=f"q16{fc}")
                        nc.vector.tensor_copy(t16[:], ps[:])
                    else:
                        t16 = kc_ring[bi % 2][fc]
                        hi = t16[0:64, :].rearrange("p (b c) -> p b c", c=128)
                        lo = t16[64:128, :].rearrange("p (b c) -> p b c", c=128)
                        nc.scalar.activation(
                            hi[:, :, 0:64],
                            ps[0:64, :].rearrange("p (b f) -> p b f", f=64),
                            AF.Copy)
                        nc.vector.tensor_copy(
                            lo[:, :, 64:128],
                            ps[64:128, :].rearrange("p (b f) -> p b f", f=64))
                    st["proj16"][name].append(t16)
                return emit

            def v_unit(mt):
                # V bias is folded host-side into qn: (P+1)(V+1b) adds
                # (F+1)*bv to every output row.
                def emit():
                    ps = ps_a.tile([128, A], F32, tag="psA")
                    for kc in range(4):
                        nc.tensor.matmul(
                            ps[:],
                            st["vT_t"][kc][:, mt * 128:(mt + 1) * 128],
                            w_sb["v"][:, kc * A:(kc + 1) * A],
                            start=(kc == 0), stop=(kc == 3))
                    v16 = v16_ring[bi % 2][mt]
                    nc.scalar.activation(
                        v16[:].rearrange("p (h c) -> p h c", c=65)[:, :, 0:64],
                        ps[:].rearrange("p (h c) -> p h c", c=64), AF.Copy)
                    st["v16_t"].append(v16)
                return emit

            units = []
            for fc in range(4):
                units.append(qk_unit("q", fc))
                units.append(qk_unit("k", fc))
            for mt in range(4):
                units.append(v_unit(mt))
            return units

        def emit_back(st, fill_units):
            """Attention + finalize for a block whose projections are done.
            fill_units (next block's projection closures) are interleaved
            between attention pairs so the PE instruction stream always has
            ready matmul work while the softmax exp runs on Scalar."""
            bi, m0 = st["bi"], st["m0"]
            proj16, v16_t, qn_t = st["proj16"], st["v16_t"], st["qn_t"]
            o_big = p_fin.tile([128, 4 * A], F16, tag="outbig")
            lg_t = {}
            fill = list(fill_units)

            def do_fill(n):
                for _ in range(n):
                    if fill:
                        fill.pop(0)()

            def do_qk(j):
                ca, cb = (2 * j) * F, (2 * j + 1) * F
                lg = ps_a.tile([128, 512], F32, tag="psA")
                for h in range(H):
                    hp, hr = h // 2, (h % 2) * 64
                    kc16 = proj16["k"][hp]
                    qc16 = proj16["q"][hp]
                    nc.tensor.matmul(
                        lg[0:64, h * 64:(h + 1) * 64],
                        kc16[:, (2 * j) * 128 + hr:(2 * j) * 128 + hr + 64],
                        qc16[:, ca:ca + 64],
                        start=True, stop=True, tile_position=(0, 0))
                    nc.tensor.matmul(
                        lg[64:128, h * 64:(h + 1) * 64],
                        kc16[:, (2 * j + 1) * 128 + hr:
                             (2 * j + 1) * 128 + hr + 64],
                        qc16[:, cb:cb + 64],
                        start=True, stop=True, tile_position=(0, 64))
                lg_t[j] = lg

            do_qk(0)
            for j in range(4):
                if j + 1 < 4:
                    do_qk(j + 1)
                lg = lg_t.pop(j)
                # exp(x - 8) -> fp16 block-diagonal over batch parity per
                # head: pt_z[:, h*128:+128] = diag(P~T(be,h), P~T(bo,h)).
                # The -8 shift keeps exp inside fp16 range (softmax is
                # shift-invariant; logits reach ~12).
                pt_z = pt_ring[(bi * 4 + j) % 3]
                hi = pt_z[0:64, :].rearrange("p (h c) -> p h c", c=128)
                lo = pt_z[64:128, :].rearrange("p (h c) -> p h c", c=128)
                nc.scalar.activation(
                    hi[:, :, 0:64],
                    lg[0:64, :].rearrange("p (h q) -> p h q", q=64), AF.Exp,
                    bias=neg8_sb[0:64, :])
                nc.scalar.activation(
                    lo[:, :, 64:128],
                    lg[64:128, :].rearrange("p (h q) -> p h q", q=64), AF.Exp,
                    bias=neg8_sb[64:128, :])
                do_fill(3)

                ov = ps_o.tile([128, 1024], F32, tag="o")
                for h in range(H):
                    nc.tensor.matmul(
                        ov[:, h * 128:h * 128 + 65],
                        pt_z[:, h * 128:(h + 1) * 128],
                        v16_t[j][:, h * 65:(h + 1) * 65],
                        start=True, stop=True)
                ovh = ov[:].rearrange("p (h c) -> p h c", c=128)
                rz = p_stat.tile([128, 8], F32, tag="rz")
                nc.vector.reciprocal(rz[:], ovh[:, :, 64])
                # normalize straight into the fp16 store tile, then add
                # colsum(V) (PSUM) on DVE, residual (SBUF) + relu
                qv = ps_a.tile([128, A], F32, tag="psA")
                nc.tensor.matmul(
                    qv[:].rearrange("p (h c) -> p h c", c=64), bcast2_sb[:],
                    v16_t[j][:].rearrange("p (h c) -> p h c", c=65)[:, :, 0:64],
                    start=True, stop=True)
                oj = o_big[:, j * A:(j + 1) * A]
                nc.vector.tensor_mul(
                    oj[:].rearrange("p (h q) -> p h q", q=64),
                    ovh[:, :, 0:64], bcast_inner(rz[:], 64))
                nc.vector.tensor_add(oj, oj, qv[:])
                nc.gpsimd.tensor_add(oj, oj, qn_t[j][:])
                nc.scalar.activation(oj, oj, AF.Relu)
                do_fill(1)

            nc.sync.dma_start(
                out.rearrange("(x p) a -> p x a", p=128)[:, m0 // 128:
                                                         m0 // 128 + 4, :],
                o_big[:].rearrange("p (x a) -> p x a", a=A))
            do_fill(99)

        st0 = emit_dmas(0)
        for u in proj_units(st0):
            u()
        prev = st0
        for bi in range(1, nblocks):
            cur = emit_dmas(bi)
            emit_back(prev, proj_units(cur))
            prev = cur
        emit_back(prev, [])

    nc.compile()
    return nc


def make_consts():
    bcast2 = np.zeros((128, 128), np.float16)
    bcast2[0:64, 0:64] = 1.0
    bcast2[64:128, 64:128] = 1.0
    return bcast2


def make_in_map(query, key, value, Wq, Wk, Wv, bv, core):
    """Build one core's input dict. query/key/value are the FULL arrays."""
    bcast2 = make_consts()
    sl = slice(core * BL, (core + 1) * BL)
    q3 = query[sl]
    xq = q3.reshape(M, D)
    xqc = (q3 - q3.mean(axis=1, keepdims=True)).reshape(M, D)
    xk = key[sl].reshape(M, D)
    xv = value[sl].reshape(M, D)
    qn_adj = xq + (F + 1) * np.asarray(bv, np.float32).reshape(1, A)
    return {
        "qT": np.ascontiguousarray(xqc.T.astype(np.float16, copy=False)),
        "kT": np.ascontiguousarray(xk.T.astype(np.float16, copy=False)),
        "vT": np.ascontiguousarray(xv.T.astype(np.float16, copy=False)),
        "qn": np.ascontiguousarray(qn_adj, dtype=np.float16),
        "wq": np.ascontiguousarray(Wq, dtype=np.float16),
        "wk": np.ascontiguousarray(Wk, dtype=np.float16),
        "wv": np.ascontiguousarray(Wv, dtype=np.float16),
        "bcast2": bcast2,
    }


_CACHED_NC = None


def kernel(query, key, value, Wq, bq, Wk, bk, Wv, bv, Wk2, bk2):
    """Full-input kernel: shards batch over 8 NeuronCores, returns full output.

    bq/bk cancel under the field-mean centering and Wk2/bk2 drop out of the
    math entirely (the unary softmax is over a size-1 axis), so they are
    accepted but unused.
    """
    global _CACHED_NC
    from concourse.bass_utils import run_bass_kernel_spmd

    query = np.asarray(query, dtype=np.float32)
    key = np.asarray(key, dtype=np.float32)
    value = np.asarray(value, dtype=np.float32)
    if _CACHED_NC is None:
        _CACHED_NC = build_program()
    in_maps = [make_in_map(query, key, value, Wq, Wk, Wv, bv, c)
               for c in range(NCORES)]
    res = run_bass_kernel_spmd(_CACHED_NC, in_maps,
                               core_ids=list(range(NCORES)), trace=False)
    parts = [res.results[c]["out"].reshape(BL, F, A).astype(np.float32)
             for c in range(NCORES)]
    return np.concatenate(parts, axis=0)

